# revision 12
# baseline (speedup 1.0000x reference)
"""Trainium2 Bass kernel for nn_Cal_Div_Loss (conv-pyramid L1 loss).

Strategy
--------
The 3x3 all-ones stride-2 VALID conv ("edgesum") is linear, so the x- and
y-pyramids collapse into a single pyramid over d = x - y.  Per sample we
need sum(d) (for the 'last' column) and sum(|d_l|) at 5 pyramid levels
(512 -> 255 -> 127 -> 63 -> 31).  The tiny cross-batch 'fuhao' sign logic
and the final mean are O(B*6) and run on the host.

Sharding: data-parallel over batch, 64 samples / 8 cores = 8 samples/core.
Per core 16 MiB of input -> DMA-bound (43.7 us measured stream time at
~390 GB/s; the kernel pipelines everything under that).

v3 pipeline (evidence-driven, vs the 93 us baseline):
  - ONE 2 MiB dma_start per sample (x and y stacked host-side) on the qSP
    HWDGE ring; banded-constant loads on gpsimd (SWDGE) so sample DMAs
    issue first.
  - d = x - y written as packed bf16 even/odd column planes (two DVE
    subtracts, sum(d) via accum_out).  All-packed-bf16 operands put the
    first colsum add into the DVE 2x_1P perf mode; the misaligned second
    add runs on the otherwise-idle GpSimd (flat 1.98 ns/elem).
  - |d| level-0 sum: single ACT Abs pass over the whole plane tile with
    accum_out.
  - level-0 row-window sums via bf16 matmuls into PSUM, emitted per
    sample; deep levels (1-4) per sample pair, deferred one pair so PE
    latency never stalls the DVE stream; |.| sums on ACT reading PSUM
    directly, window sums on DVE (no SBUF evacuation copies at all).
"""

import sys

if "/opt/trn_rl_repo" not in sys.path:
    sys.path.insert(0, "/opt/trn_rl_repo")

import numpy as np

# ---------------------------------------------------------------- constants
B = 64          # full batch
NCORES = 8
S = B // NCORES  # samples per core
P = 128
N0, N1, N2, N3, N4 = 512, 255, 127, 63, 31
G0 = 4          # row-block chunks at level 0 (row = g*128 + p)
LAYER_NUM = 4

# stats_a columns: [0:8] sa0, [8:16] sa1 (d1 rows 0..127),
# [16:24] sa1 (d1 rows 128..254, parts 0..126), [24:32] sa2 (parts 0..126),
# [32:40] sa3 (parts 0..62), [40:48] sa4 (parts 0..30)
ACT_COLS = 48

_CACHE = {}


def _banded(n_out, n_in, pad_in_to=None, pad_out_to=None):
    """R^T for the window-3 stride-2 row sum: [n_in(, padded), n_out] bf16."""
    import ml_dtypes

    r = np.zeros((n_out, n_in), dtype=np.float32)
    for i in range(n_out):
        r[i, 2 * i : 2 * i + 3] = 1.0
    bt = np.ascontiguousarray(r.T)
    if pad_in_to is not None and pad_in_to > n_in:
        bt = np.concatenate(
            [bt, np.zeros((pad_in_to - n_in, n_out), dtype=np.float32)], axis=0
        )
    if pad_out_to is not None and pad_out_to > n_out:
        bt = np.concatenate(
            [bt, np.zeros((bt.shape[0], pad_out_to - n_out), dtype=np.float32)],
            axis=1,
        )
    return bt.astype(ml_dtypes.bfloat16)


def _build_nc():
    from contextlib import ExitStack

    import concourse.bacc as bacc
    import concourse.mybir as mybir
    import concourse.tile as tile

    f32 = mybir.dt.float32
    bf16 = mybir.dt.bfloat16
    SUB = mybir.AluOpType.subtract
    ADD = mybir.AluOpType.add
    AF = mybir.ActivationFunctionType

    nc = bacc.Bacc("TRN2", target_bir_lowering=False, debug=False)
    xys = nc.dram_tensor("xys", [S, 2, 512, 512], f32, kind="ExternalInput").ap()
    bt0 = nc.dram_tensor("bt0", [512, 256], bf16, kind="ExternalInput").ap()
    bt1 = nc.dram_tensor("bt1", [256, 128], bf16, kind="ExternalInput").ap()
    bt2 = nc.dram_tensor("bt2", [N2, 64], bf16, kind="ExternalInput").ap()
    bt3 = nc.dram_tensor("bt3", [N3, 32], bf16, kind="ExternalInput").ap()
    sd_out = nc.dram_tensor("sd", [P, 2 * S], f32, kind="ExternalOutput").ap()
    sa_out = nc.dram_tensor("sa", [P, ACT_COLS], f32, kind="ExternalOutput").ap()

    with tile.TileContext(nc) as tc, ExitStack() as ctx:
        singles = ctx.enter_context(tc.tile_pool(name="singles", bufs=1))
        xy = ctx.enter_context(tc.tile_pool(name="xy", bufs=4))
        dpool = ctx.enter_context(tc.tile_pool(name="d", bufs=2))
        vpool = ctx.enter_context(tc.tile_pool(name="v", bufs=4))
        upool = ctx.enter_context(tc.tile_pool(name="u", bufs=2))
        scr = ctx.enter_context(tc.tile_pool(name="scr", bufs=2))
        psA = ctx.enter_context(tc.tile_pool(name="psA", bufs=2, space="PSUM"))
        psum = ctx.enter_context(tc.tile_pool(name="ps", bufs=1, space="PSUM"))

        # banded constants via SWDGE (off the critical qSP ring)
        bt0_sb = singles.tile([P, G0, 256], bf16)
        nc.scalar.dma_start(out=bt0_sb, in_=bt0.rearrange("(p g) i -> p g i", g=G0))
        bt1_sb = singles.tile([P, 2, 128], bf16)
        nc.scalar.dma_start(out=bt1_sb, in_=bt1.rearrange("(g p) i -> p g i", p=P))
        bt2_sb = singles.tile([N2, 64], bf16)
        nc.scalar.dma_start(out=bt2_sb, in_=bt2)
        bt3_sb = singles.tile([N3, 32], bf16)
        nc.scalar.dma_start(out=bt3_sb, in_=bt3)

        stats_d = singles.tile([P, 2 * S], f32)    # sd halves via DVE accum
        stats_a = singles.tile([P, ACT_COLS], f32)  # |.| sums via ACT accum
        nc.vector.memset(stats_d, 0.0)
        nc.scalar.memzero(stats_a)

        EVEN = slice(0, 512, 2)

        def colsum_dve(v_out, src, n_out, tag):
            """v_out[..., i] = src[..., 2i]+src[..., 2i+1]+src[..., 2i+2]
            on DVE for a PSUM source.  tensor_tensor may read only ONE
            operand from PSUM, so the odd columns are evacuated to SBUF
            first and each add pairs one PSUM stream with one SBUF one."""
            ps_shape = list(src.shape[:-1]) + [n_out]
            sl = [slice(None)] * (len(src.shape) - 1)
            e0 = src[tuple(sl + [slice(0, 2 * n_out - 1, 2)])]
            e1 = src[tuple(sl + [slice(1, 2 * n_out, 2)])]
            e2 = src[tuple(sl + [slice(2, 2 * n_out + 1, 2)])]
            odd = upool.tile(ps_shape, bf16, tag=f"o{tag}")
            with nc.allow_low_precision(reason="2e-2 tolerance; bf16 pyramid"):
                nc.vector.tensor_copy(out=odd, in_=e1)
            u = upool.tile(ps_shape, bf16, tag=f"u{tag}")
            nc.vector.tensor_add(out=u, in0=e0, in1=odd)
            nc.vector.tensor_add(out=v_out, in0=u, in1=e2)

        pend = {}

        def deep_chain(pr):
            """Levels 1-4 for pair pr, reading level-0 matmuls from PSUM."""
            pdA, pdB = pend.pop(pr)
            s0 = 2 * pr
            # |d1| sums on ACT (per sample, per row-block)
            asc1 = scr.tile([P, 2, 256], bf16, tag="asc1")
            for j in range(2):
                nc.scalar.activation(
                    out=asc1[:, j, 0:N1], in_=pdA[:, j, 0:N1], func=AF.Abs,
                    accum_out=stats_a[:, 8 + s0 + j : 9 + s0 + j],
                )
                nc.scalar.activation(
                    out=asc1[0:127, j, 0:N1], in_=pdB[:, j, 0:N1], func=AF.Abs,
                    accum_out=stats_a[0:127, 16 + s0 + j : 17 + s0 + j],
                )
            # level-1 column sums (DVE, PSUM f32 in -> bf16 out)
            v1a = vpool.tile([P, 2, 128], bf16, tag="v1a")
            v1b = vpool.tile([127, 2, 128], bf16, tag="v1b")
            colsum_dve(v1a[:, :, 0:N2], pdA[:, :, 0:N1], N2, "1a")
            colsum_dve(v1b[:, :, 0:N2], pdB[:, :, 0:N1], N2, "1b")

            # level-2 matmul: d2 = R1 @ v1
            wb2 = psum.tile([N2, 2, 128], f32, tag="wb2")
            for j in range(2):
                nc.tensor.matmul(
                    wb2[:, j, 0:N2], bt1_sb[:, 0, 0:N2], v1a[:, j, 0:N2],
                    start=True, stop=False,
                )
                nc.tensor.matmul(
                    wb2[:, j, 0:N2], bt1_sb[0:127, 1, 0:N2], v1b[:, j, 0:N2],
                    start=False, stop=True,
                )
            asc2 = scr.tile([N2, 2, 128], bf16, tag="asc2")
            for j in range(2):
                nc.scalar.activation(
                    out=asc2[:, j, 0:N2], in_=wb2[:, j, 0:N2], func=AF.Abs,
                    accum_out=stats_a[0:N2, 24 + s0 + j : 25 + s0 + j],
                )
            v2 = vpool.tile([N2, 2, 64], bf16, tag="v2")
            colsum_dve(v2[:, :, 0:N3], wb2[:, :, 0:N2], N3, "2")

            # level-3
            wb3 = psum.tile([N3, 2, 64], f32, tag="wb3")
            for j in range(2):
                nc.tensor.matmul(
                    wb3[:, j, 0:N3], bt2_sb[:, 0:N3], v2[:, j, 0:N3],
                    start=True, stop=True,
                )
            with nc.allow_low_precision(reason="f32 accum via tensor_reduce"):
                nc.vector.tensor_reduce(
                    out=stats_a[0:N3, 32 + s0 : 34 + s0], in_=wb3[:, :, 0:N3],
                    axis=mybir.AxisListType.X, op=ADD,
                    apply_absolute_value=True,
                )
            v3 = vpool.tile([N3, 2, 32], bf16, tag="v3")
            colsum_dve(v3[:, :, 0:N4], wb3[:, :, 0:N3], N4, "3")

            # level-4
            wb4 = psum.tile([N4, 2, 32], f32, tag="wb4")
            for j in range(2):
                nc.tensor.matmul(
                    wb4[:, j, 0:N4], bt3_sb[:, 0:N4], v3[:, j, 0:N4],
                    start=True, stop=True,
                )
            with nc.allow_low_precision(reason="f32 accum via tensor_reduce"):
                nc.vector.tensor_reduce(
                    out=stats_a[0:N4, 40 + s0 : 42 + s0], in_=wb4[:, :, 0:N4],
                    axis=mybir.AxisListType.X, op=ADD,
                    apply_absolute_value=True,
                )

        xyts = {}

        def issue_dma(s):
            xyt = xy.tile([P, 2, G0, N0], f32, tag="xyt")
            eng = nc.sync if s % 2 == 0 else nc.gpsimd
            eng.dma_start(
                out=xyt, in_=xys[s].rearrange("t (p g) c -> p t g c", g=G0)
            )
            xyts[s] = xyt

        PF = 4  # prefetch depth == xy pool bufs
        for s in range(PF):
            issue_dma(s)

        for s in range(S):
            if s + PF < S:
                issue_dma(s + PF)
            xyt = xyts.pop(s)

            if s % 2 == 0:
                pdA = psA.tile([P, 2, 256], f32, tag="pdA")
                pdB = psA.tile([127, 2, 256], f32, tag="pdB")
                pend[s // 2] = (pdA, pdB)

            # d = x - y as packed bf16 even/odd planes; signed sums -> stats_d
            dt = dpool.tile([P, G0, 2, 256], bf16, tag="dt")
            nc.vector.scalar_tensor_tensor(
                out=dt[:, :, 0, :], in0=xyt[:, 0, :, EVEN], scalar=0.0,
                in1=xyt[:, 1, :, EVEN], op0=ADD, op1=SUB,
                accum_out=stats_d[:, 2 * s : 2 * s + 1],
            )
            ODD = slice(1, 512, 2)
            nc.vector.scalar_tensor_tensor(
                out=dt[:, :, 1, :], in0=xyt[:, 0, :, ODD], scalar=0.0,
                in1=xyt[:, 1, :, ODD], op0=ADD, op1=SUB,
                accum_out=stats_d[:, 2 * s + 1 : 2 * s + 2],
            )

            # sum |d| on ACT: one pass over both planes
            ascr = scr.tile([P, G0, 2, 256], bf16, tag="ascr")
            nc.scalar.activation(
                out=ascr, in_=dt, func=AF.Abs,
                accum_out=stats_a[:, s : s + 1],
            )

            # level-0 column-window sum:
            #   u = e + o (DVE 2x packed), v0 = u + e[1:] (GpSimd)
            u0 = upool.tile([P, G0, 256], bf16, tag="u0")
            nc.vector.tensor_add(
                out=u0[:, :, 0:N1], in0=dt[:, :, 0, 0:N1], in1=dt[:, :, 1, 0:N1]
            )
            v0 = vpool.tile([P, G0, 256], bf16, tag="v0")
            if s < S - 2:
                nc.gpsimd.tensor_add(
                    out=v0[:, :, 0:N1], in0=u0[:, :, 0:N1], in1=dt[:, :, 0, 1:256]
                )
            else:
                # tail samples: keep the critical path off the slow Q7
                nc.vector.tensor_add(
                    out=v0[:, :, 0:N1], in0=u0[:, :, 0:N1], in1=dt[:, :, 0, 1:256]
                )

            # level-0 row-window matmuls, emitted per sample.
            # Row r = 4p+g: output block m=0 (rows 0..127) draws from
            # partitions [0:65] (g=0) / [0:64] (g>0); m=1 (rows 128..254)
            # from partitions [64:128] on every chunk.
            pdA, pdB = pend[s // 2]
            for w, mp, m in ((pdA, P, 0), (pdB, 127, 1)):
                for g in range(G0):
                    if m == 0:
                        lo, hi = 0, (65 if g == 0 else 64)
                    else:
                        lo, hi = 64, 128
                    nc.tensor.matmul(
                        w[:, s % 2, 0:N1],
                        bt0_sb[lo:hi, g, m * 128 : m * 128 + mp],
                        v0[lo:hi, g, 0:N1],
                        start=(g == 0),
                        stop=(g == G0 - 1),
                    )

            # deep-chain one pair behind so PE latency never blocks DVE
            if s % 2 == 1 and s >= 3:
                deep_chain(s // 2 - 1)

        deep_chain(S // 2 - 1)

        nc.sync.dma_start(out=sd_out, in_=stats_d)
        nc.sync.dma_start(out=sa_out, in_=stats_a)

    nc.finalize()
    return nc


def _get_nc():
    if "nc" not in _CACHE:
        _CACHE["nc"] = _build_nc()
    return _CACHE["nc"]


def _run_on_hw(x, y, trace=False):
    """x, y: [64, 512, 512] fp32 numpy. Returns list of 8 (sd, sa) pairs."""
    from concourse.bass_utils import run_bass_kernel_spmd

    nc = _get_nc()
    bt0 = _banded(N1, 512, pad_out_to=256)
    bt1 = _banded(N2, N1, pad_in_to=256, pad_out_to=128)
    bt2 = _banded(N3, N2, pad_out_to=64)
    bt3 = _banded(N4, N3, pad_out_to=32)

    in_maps = []
    for c in range(NCORES):
        xc = x[c * S : (c + 1) * S]
        yc = y[c * S : (c + 1) * S]
        in_maps.append(
            {
                "xys": np.ascontiguousarray(np.stack([xc, yc], axis=1)),
                "bt0": bt0,
                "bt1": bt1,
                "bt2": bt2,
                "bt3": bt3,
            }
        )

    res = run_bass_kernel_spmd(
        nc, in_maps, core_ids=list(range(NCORES)), trace=trace
    )
    _CACHE["last_results"] = res
    return [(r["sd"], r["sa"]) for r in res.results]


def kernel(x, y, alpha, _trace=False):
    x = np.ascontiguousarray(np.asarray(x, dtype=np.float32).reshape(B, 512, 512))
    y = np.ascontiguousarray(np.asarray(y, dtype=np.float32).reshape(B, 512, 512))
    alpha = np.asarray(alpha, dtype=np.float32)

    stats_list = _run_on_hw(x, y, trace=_trace)

    sd = np.empty(B, np.float64)
    sa = np.empty((B, 5), np.float64)
    for c in range(NCORES):
        std, sta = stats_list[c]
        std = std.astype(np.float64)
        sta = sta.astype(np.float64)
        for s in range(S):
            b = c * S + s
            sd[b] = std[:, 2 * s].sum() + std[:, 2 * s + 1].sum()
            sa[b, 0] = sta[:, s].sum()
            sa[b, 1] = sta[:, 8 + s].sum() + sta[0:127, 16 + s].sum()
            sa[b, 2] = sta[0:N2, 24 + s].sum()
            sa[b, 3] = sta[0:N3, 32 + s].sum()
            sa[b, 4] = sta[0:N4, 40 + s].sum()

    counts = np.array(
        [N0 * N0, N1 * N1, N2 * N2, N3 * N3, N4 * N4], np.float64
    )
    l1 = sa / counts  # [B, 5]
    last = np.abs(sd) * float(LAYER_NUM + 1)  # [B]

    # faithful 'fuhao' replication (matches reference.py exactly)
    k_layer = (alpha * np.float32(LAYER_NUM + 2)).astype(np.int32)  # [B]
    trig = k_layer <= LAYER_NUM
    triggered_before = np.concatenate(
        [np.zeros(1, bool), np.cumsum(trig)[:-1] > 0]
    )
    i_idx = np.arange(LAYER_NUM + 1)
    sign = np.where(
        triggered_before[:, None] | (i_idx[None, :] >= k_layer[:, None]),
        1.0,
        -1.0,
    )

    loss_tensor = np.concatenate([l1 * sign, last[:, None]], axis=1)
    return np.float32(loss_tensor.mean())


# revision 15
# speedup vs baseline: 1.0087x; 1.0087x over previous
"""Trainium2 Bass kernel for nn_Cal_Div_Loss (conv-pyramid L1 loss).

Strategy
--------
The 3x3 all-ones stride-2 VALID conv ("edgesum") is linear, so the x- and
y-pyramids collapse into a single pyramid over d = x - y.  Per sample we
need sum(d) (for the 'last' column) and sum(|d_l|) at 5 pyramid levels
(512 -> 255 -> 127 -> 63 -> 31).  The tiny cross-batch 'fuhao' sign logic
and the final mean are O(B*6) and run on the host.

Sharding: data-parallel over batch, 64 samples / 8 cores = 8 samples/core.
Per core 16 MiB of input -> DMA-bound (43.7 us measured stream time at
~390 GB/s; the kernel pipelines everything under that).

v3 pipeline (evidence-driven, vs the 93 us baseline):
  - ONE 2 MiB dma_start per sample (x and y stacked host-side) on the qSP
    HWDGE ring; banded-constant loads on gpsimd (SWDGE) so sample DMAs
    issue first.
  - d = x - y written as packed bf16 even/odd column planes (two DVE
    subtracts, sum(d) via accum_out).  All-packed-bf16 operands put the
    first colsum add into the DVE 2x_1P perf mode; the misaligned second
    add runs on the otherwise-idle GpSimd (flat 1.98 ns/elem).
  - |d| level-0 sum: single ACT Abs pass over the whole plane tile with
    accum_out.
  - level-0 row-window sums via bf16 matmuls into PSUM, emitted per
    sample; deep levels (1-4) per sample pair, deferred one pair so PE
    latency never stalls the DVE stream; |.| sums on ACT reading PSUM
    directly, window sums on DVE (no SBUF evacuation copies at all).
"""

import sys

if "/opt/trn_rl_repo" not in sys.path:
    sys.path.insert(0, "/opt/trn_rl_repo")

import numpy as np

# ---------------------------------------------------------------- constants
B = 64          # full batch
NCORES = 8
S = B // NCORES  # samples per core
P = 128
N0, N1, N2, N3, N4 = 512, 255, 127, 63, 31
G0 = 4          # row-block chunks at level 0 (row = g*128 + p)
LAYER_NUM = 4

# stats_a columns: [0:8] sa0, [8:16] sa1 (d1 rows 0..127),
# [16:24] sa1 (d1 rows 128..254, parts 0..126), [24:32] sa2 (parts 0..126),
# [32:40] sa3 (parts 0..62), [40:48] sa4 (parts 0..30)
ACT_COLS = 48

_CACHE = {}


def _banded(n_out, n_in, pad_in_to=None, pad_out_to=None):
    """R^T for the window-3 stride-2 row sum: [n_in(, padded), n_out] bf16."""
    import ml_dtypes

    r = np.zeros((n_out, n_in), dtype=np.float32)
    for i in range(n_out):
        r[i, 2 * i : 2 * i + 3] = 1.0
    bt = np.ascontiguousarray(r.T)
    if pad_in_to is not None and pad_in_to > n_in:
        bt = np.concatenate(
            [bt, np.zeros((pad_in_to - n_in, n_out), dtype=np.float32)], axis=0
        )
    if pad_out_to is not None and pad_out_to > n_out:
        bt = np.concatenate(
            [bt, np.zeros((bt.shape[0], pad_out_to - n_out), dtype=np.float32)],
            axis=1,
        )
    return bt.astype(ml_dtypes.bfloat16)


def _build_nc():
    from contextlib import ExitStack

    import concourse.bacc as bacc
    import concourse.mybir as mybir
    import concourse.tile as tile

    f32 = mybir.dt.float32
    bf16 = mybir.dt.bfloat16
    SUB = mybir.AluOpType.subtract
    ADD = mybir.AluOpType.add
    AF = mybir.ActivationFunctionType

    nc = bacc.Bacc("TRN2", target_bir_lowering=False, debug=False)
    xs = nc.dram_tensor("xs", [S, 512, 512], f32, kind="ExternalInput").ap()
    ys = nc.dram_tensor("ys", [S, 512, 512], f32, kind="ExternalInput").ap()
    bt0 = nc.dram_tensor("bt0", [512, 256], bf16, kind="ExternalInput").ap()
    bt1 = nc.dram_tensor("bt1", [256, 128], bf16, kind="ExternalInput").ap()
    bt2 = nc.dram_tensor("bt2", [N2, 64], bf16, kind="ExternalInput").ap()
    bt3 = nc.dram_tensor("bt3", [N3, 32], bf16, kind="ExternalInput").ap()
    sd_out = nc.dram_tensor("sd", [P, 2 * S], f32, kind="ExternalOutput").ap()
    sa_out = nc.dram_tensor("sa", [P, ACT_COLS], f32, kind="ExternalOutput").ap()

    with tile.TileContext(nc) as tc, ExitStack() as ctx:
        singles = ctx.enter_context(tc.tile_pool(name="singles", bufs=1))
        xp = ctx.enter_context(tc.tile_pool(name="xp", bufs=8))
        yp = ctx.enter_context(tc.tile_pool(name="yp", bufs=8))
        dpool = ctx.enter_context(tc.tile_pool(name="d", bufs=2))
        vpool = ctx.enter_context(tc.tile_pool(name="v", bufs=4))
        upool = ctx.enter_context(tc.tile_pool(name="u", bufs=2))
        scr = ctx.enter_context(tc.tile_pool(name="scr", bufs=2))
        psA = ctx.enter_context(tc.tile_pool(name="psA", bufs=2, space="PSUM"))
        psum = ctx.enter_context(tc.tile_pool(name="ps", bufs=1, space="PSUM"))

        # all input DMAs issue upfront: x halves on the qSP ring, y halves
        # on the qAct ring — two descriptor streams, samples complete in
        # order, no buffer waits (bufs=8 holds the full working set)
        xts, yts = {}, {}
        for s_ in range(S):
            xts[s_] = xp.tile([P, G0, N0], f32, tag="xt", name=f"xt{s_}")
            nc.sync.dma_start(
                out=xts[s_], in_=xs[s_].rearrange("(p g) c -> p g c", g=G0)
            )
        for s_ in range(S):
            yts[s_] = yp.tile([P, G0, N0], f32, tag="yt", name=f"yt{s_}")
            nc.scalar.dma_start(
                out=yts[s_], in_=ys[s_].rearrange("(p g) c -> p g c", g=G0)
            )
        # banded constants after the inputs on qSP (tiny; needed from ~20us)
        bt0_sb = singles.tile([P, G0, 256], bf16)
        nc.sync.dma_start(out=bt0_sb, in_=bt0.rearrange("(p g) i -> p g i", g=G0))
        bt1_sb = singles.tile([P, 2, 128], bf16)
        nc.sync.dma_start(out=bt1_sb, in_=bt1.rearrange("(g p) i -> p g i", p=P))
        bt2_sb = singles.tile([N2, 64], bf16)
        nc.sync.dma_start(out=bt2_sb, in_=bt2)
        bt3_sb = singles.tile([N3, 32], bf16)
        nc.sync.dma_start(out=bt3_sb, in_=bt3)

        stats_d = singles.tile([P, 2 * S], f32)    # sd halves via DVE accum
        stats_a = singles.tile([P, ACT_COLS], f32)  # |.| sums via ACT accum
        nc.vector.memset(stats_d, 0.0)
        nc.scalar.memzero(stats_a)

        EVEN = slice(0, 512, 2)

        def colsum_dve(v_out, src, n_out, tag):
            """v_out[..., i] = src[..., 2i]+src[..., 2i+1]+src[..., 2i+2]
            on DVE for a PSUM source.  tensor_tensor may read only ONE
            operand from PSUM, so the odd columns are evacuated to SBUF
            first and each add pairs one PSUM stream with one SBUF one."""
            ps_shape = list(src.shape[:-1]) + [n_out]
            sl = [slice(None)] * (len(src.shape) - 1)
            e0 = src[tuple(sl + [slice(0, 2 * n_out - 1, 2)])]
            e1 = src[tuple(sl + [slice(1, 2 * n_out, 2)])]
            e2 = src[tuple(sl + [slice(2, 2 * n_out + 1, 2)])]
            odd = upool.tile(ps_shape, bf16, tag=f"o{tag}")
            with nc.allow_low_precision(reason="2e-2 tolerance; bf16 pyramid"):
                nc.vector.tensor_copy(out=odd, in_=e1)
            u = upool.tile(ps_shape, bf16, tag=f"u{tag}")
            nc.vector.tensor_add(out=u, in0=e0, in1=odd)
            nc.vector.tensor_add(out=v_out, in0=u, in1=e2)

        pend = {}

        def deep_chain(pr):
            """Levels 1-4 for pair pr, reading level-0 matmuls from PSUM."""
            pdA, pdB = pend.pop(pr)
            s0 = 2 * pr
            # |d1| sums on ACT (per sample, per row-block)
            asc1 = scr.tile([P, 2, 256], bf16, tag="asc1")
            for j in range(2):
                nc.scalar.activation(
                    out=asc1[:, j, 0:N1], in_=pdA[:, j, 0:N1], func=AF.Abs,
                    accum_out=stats_a[:, 8 + s0 + j : 9 + s0 + j],
                )
                nc.scalar.activation(
                    out=asc1[0:127, j, 0:N1], in_=pdB[:, j, 0:N1], func=AF.Abs,
                    accum_out=stats_a[0:127, 16 + s0 + j : 17 + s0 + j],
                )
            # level-1 column sums (DVE, PSUM f32 in -> bf16 out)
            v1a = vpool.tile([P, 2, 128], bf16, tag="v1a")
            v1b = vpool.tile([127, 2, 128], bf16, tag="v1b")
            colsum_dve(v1a[:, :, 0:N2], pdA[:, :, 0:N1], N2, "1a")
            colsum_dve(v1b[:, :, 0:N2], pdB[:, :, 0:N1], N2, "1b")

            # level-2 matmul: d2 = R1 @ v1
            wb2 = psum.tile([N2, 2, 128], f32, tag="wb2")
            for j in range(2):
                nc.tensor.matmul(
                    wb2[:, j, 0:N2], bt1_sb[:, 0, 0:N2], v1a[:, j, 0:N2],
                    start=True, stop=False,
                )
                nc.tensor.matmul(
                    wb2[:, j, 0:N2], bt1_sb[0:127, 1, 0:N2], v1b[:, j, 0:N2],
                    start=False, stop=True,
                )
            asc2 = scr.tile([N2, 2, 128], bf16, tag="asc2")
            for j in range(2):
                nc.scalar.activation(
                    out=asc2[:, j, 0:N2], in_=wb2[:, j, 0:N2], func=AF.Abs,
                    accum_out=stats_a[0:N2, 24 + s0 + j : 25 + s0 + j],
                )
            v2 = vpool.tile([N2, 2, 64], bf16, tag="v2")
            colsum_dve(v2[:, :, 0:N3], wb2[:, :, 0:N2], N3, "2")

            # level-3
            wb3 = psum.tile([N3, 2, 64], f32, tag="wb3")
            for j in range(2):
                nc.tensor.matmul(
                    wb3[:, j, 0:N3], bt2_sb[:, 0:N3], v2[:, j, 0:N3],
                    start=True, stop=True,
                )
            nc.vector.tensor_reduce(
                out=stats_a[0:N3, 32 + s0 : 34 + s0], in_=wb3[:, :, 0:N3],
                axis=mybir.AxisListType.X, op=ADD, apply_absolute_value=True,
            )
            v3 = vpool.tile([N3, 2, 32], bf16, tag="v3")
            colsum_dve(v3[:, :, 0:N4], wb3[:, :, 0:N3], N4, "3")

            # level-4
            wb4 = psum.tile([N4, 2, 32], f32, tag="wb4")
            for j in range(2):
                nc.tensor.matmul(
                    wb4[:, j, 0:N4], bt3_sb[:, 0:N4], v3[:, j, 0:N4],
                    start=True, stop=True,
                )
            nc.vector.tensor_reduce(
                out=stats_a[0:N4, 40 + s0 : 42 + s0], in_=wb4[:, :, 0:N4],
                axis=mybir.AxisListType.X, op=ADD, apply_absolute_value=True,
            )

        v0s = {}
        for s in range(S):
            xt, yt = xts[s], yts[s]

            if s % 2 == 0:
                pdA = psA.tile([P, 2, 256], f32, tag="pdA")
                pdB = psA.tile([127, 2, 256], f32, tag="pdB")
                pend[s // 2] = (pdA, pdB)

            # d = x - y as packed bf16 even/odd planes; signed sums -> stats_d
            dt = dpool.tile([P, G0, 2, 256], bf16, tag="dt")
            ODD = slice(1, 512, 2)
            nc.vector.scalar_tensor_tensor(
                out=dt[:, :, 0, :], in0=xt[:, :, EVEN], scalar=0.0,
                in1=yt[:, :, EVEN], op0=ADD, op1=SUB,
                accum_out=stats_d[:, 2 * s : 2 * s + 1],
            )
            nc.vector.scalar_tensor_tensor(
                out=dt[:, :, 1, :], in0=xt[:, :, ODD], scalar=0.0,
                in1=yt[:, :, ODD], op0=ADD, op1=SUB,
                accum_out=stats_d[:, 2 * s + 1 : 2 * s + 2],
            )

            # sum |d| on ACT: one pass over both planes
            ascr = scr.tile([P, G0, 2, 256], bf16, tag="ascr")
            nc.scalar.activation(
                out=ascr, in_=dt, func=AF.Abs,
                accum_out=stats_a[:, s : s + 1],
            )

            # level-0 column-window sum:
            #   u = e + o (DVE 2x packed), v0 = u + e[1:] (GpSimd)
            u0 = upool.tile([P, G0, 256], bf16, tag="u0")
            nc.vector.tensor_add(
                out=u0[:, :, 0:N1], in0=dt[:, :, 0, 0:N1], in1=dt[:, :, 1, 0:N1]
            )
            v0 = vpool.tile([P, G0, 256], bf16, tag="v0")
            if s < S - 2:
                nc.gpsimd.tensor_add(
                    out=v0[:, :, 0:N1], in0=u0[:, :, 0:N1], in1=dt[:, :, 0, 1:256]
                )
            else:
                # tail samples: keep the critical path off the slow Q7
                nc.vector.tensor_add(
                    out=v0[:, :, 0:N1], in0=u0[:, :, 0:N1], in1=dt[:, :, 0, 1:256]
                )

            # level-0 row-window matmuls.  Row r = 4p+g: block m=0 (rows
            # 0..127) draws from partitions [0:65] (g=0) / [0:64] (g>0);
            # m=1 (rows 128..254) from partitions [64:128] on every chunk.
            v0s[s] = v0
            if s % 2 == 1 or s >= S - 2:
                pdA, pdB = pend[s // 2]
                emit = (s - 1, s) if (s % 2 == 1 and s < S - 2) else (s,)
                for w, mp, m in ((pdA, P, 0), (pdB, 127, 1)):
                    for g in range(G0):
                        if m == 0:
                            lo, hi = 0, (65 if g == 0 else 64)
                        else:
                            lo, hi = 64, 128
                        for sp in emit:
                            nc.tensor.matmul(
                                w[:, sp % 2, 0:N1],
                                bt0_sb[lo:hi, g, m * 128 : m * 128 + mp],
                                v0s[sp][lo:hi, g, 0:N1],
                                start=(g == 0),
                                stop=(g == G0 - 1),
                            )

            # deep-chain one pair behind so PE latency never blocks DVE
            if s % 2 == 1 and s >= 3:
                deep_chain(s // 2 - 1)

        deep_chain(S // 2 - 1)

        nc.sync.dma_start(out=sd_out, in_=stats_d)
        nc.sync.dma_start(out=sa_out, in_=stats_a)

    nc.finalize()
    return nc


def _get_nc():
    if "nc" not in _CACHE:
        _CACHE["nc"] = _build_nc()
    return _CACHE["nc"]


def _run_on_hw(x, y, trace=False):
    """x, y: [64, 512, 512] fp32 numpy. Returns list of 8 (sd, sa) pairs."""
    from concourse.bass_utils import run_bass_kernel_spmd

    nc = _get_nc()
    bt0 = _banded(N1, 512, pad_out_to=256)
    bt1 = _banded(N2, N1, pad_in_to=256, pad_out_to=128)
    bt2 = _banded(N3, N2, pad_out_to=64)
    bt3 = _banded(N4, N3, pad_out_to=32)

    in_maps = []
    for c in range(NCORES):
        xc = x[c * S : (c + 1) * S]
        yc = y[c * S : (c + 1) * S]
        in_maps.append(
            {
                "xs": np.ascontiguousarray(xc),
                "ys": np.ascontiguousarray(yc),
                "bt0": bt0,
                "bt1": bt1,
                "bt2": bt2,
                "bt3": bt3,
            }
        )

    res = run_bass_kernel_spmd(
        nc, in_maps, core_ids=list(range(NCORES)), trace=trace
    )
    _CACHE["last_results"] = res
    return [(r["sd"], r["sa"]) for r in res.results]


def kernel(x, y, alpha, _trace=False):
    x = np.ascontiguousarray(np.asarray(x, dtype=np.float32).reshape(B, 512, 512))
    y = np.ascontiguousarray(np.asarray(y, dtype=np.float32).reshape(B, 512, 512))
    alpha = np.asarray(alpha, dtype=np.float32)

    stats_list = _run_on_hw(x, y, trace=_trace)

    sd = np.empty(B, np.float64)
    sa = np.empty((B, 5), np.float64)
    for c in range(NCORES):
        std, sta = stats_list[c]
        std = std.astype(np.float64)
        sta = sta.astype(np.float64)
        for s in range(S):
            b = c * S + s
            sd[b] = std[:, 2 * s].sum() + std[:, 2 * s + 1].sum()
            sa[b, 0] = sta[:, s].sum()
            sa[b, 1] = sta[:, 8 + s].sum() + sta[0:127, 16 + s].sum()
            sa[b, 2] = sta[0:N2, 24 + s].sum()
            sa[b, 3] = sta[0:N3, 32 + s].sum()
            sa[b, 4] = sta[0:N4, 40 + s].sum()

    counts = np.array(
        [N0 * N0, N1 * N1, N2 * N2, N3 * N3, N4 * N4], np.float64
    )
    l1 = sa / counts  # [B, 5]
    last = np.abs(sd) * float(LAYER_NUM + 1)  # [B]

    # faithful 'fuhao' replication (matches reference.py exactly)
    k_layer = (alpha * np.float32(LAYER_NUM + 2)).astype(np.int32)  # [B]
    trig = k_layer <= LAYER_NUM
    triggered_before = np.concatenate(
        [np.zeros(1, bool), np.cumsum(trig)[:-1] > 0]
    )
    i_idx = np.arange(LAYER_NUM + 1)
    sign = np.where(
        triggered_before[:, None] | (i_idx[None, :] >= k_layer[:, None]),
        1.0,
        -1.0,
    )

    loss_tensor = np.concatenate([l1 * sign, last[:, None]], axis=1)
    return np.float32(loss_tensor.mean())


# revision 16
# speedup vs baseline: 1.2252x; 1.2146x over previous
"""Trainium2 Bass kernel for nn_Cal_Div_Loss (conv-pyramid L1 loss).

Strategy
--------
The 3x3 all-ones stride-2 VALID conv ("edgesum") is linear, so the x- and
y-pyramids collapse into a single pyramid over d = x - y.  Per sample we
need sum(d) (for the 'last' column) and sum(|d_l|) at 5 pyramid levels
(512 -> 255 -> 127 -> 63 -> 31).  The tiny cross-batch 'fuhao' sign logic
and the final mean are O(B*6) and run on the host.

Sharding: data-parallel over batch, 64 samples / 8 cores = 8 samples/core.
Per core 16 MiB of input -> DMA-bound (43.7 us measured stream time at
~390 GB/s; the kernel pipelines everything under that).

v3 pipeline (evidence-driven, vs the 93 us baseline):
  - ONE 2 MiB dma_start per sample (x and y stacked host-side) on the qSP
    HWDGE ring; banded-constant loads on gpsimd (SWDGE) so sample DMAs
    issue first.
  - d = x - y written as packed bf16 even/odd column planes (two DVE
    subtracts, sum(d) via accum_out).  All-packed-bf16 operands put the
    first colsum add into the DVE 2x_1P perf mode; the misaligned second
    add runs on the otherwise-idle GpSimd (flat 1.98 ns/elem).
  - |d| level-0 sum: single ACT Abs pass over the whole plane tile with
    accum_out.
  - level-0 row-window sums via bf16 matmuls into PSUM, emitted per
    sample; deep levels (1-4) per sample pair, deferred one pair so PE
    latency never stalls the DVE stream; |.| sums on ACT reading PSUM
    directly, window sums on DVE (no SBUF evacuation copies at all).
"""

import sys

if "/opt/trn_rl_repo" not in sys.path:
    sys.path.insert(0, "/opt/trn_rl_repo")

import numpy as np

# ---------------------------------------------------------------- constants
B = 64          # full batch
NCORES = 8
S = B // NCORES  # samples per core
P = 128
N0, N1, N2, N3, N4 = 512, 255, 127, 63, 31
G0 = 4          # row-block chunks at level 0 (row = g*128 + p)
LAYER_NUM = 4

# stats_a columns: [0:8] sa0, [8:16] sa1 (d1 rows 0..127),
# [16:24] sa1 (d1 rows 128..254, parts 0..126), [24:32] sa2 (parts 0..126),
# [32:40] sa3 (parts 0..62), [40:48] sa4 (parts 0..30)
ACT_COLS = 48

_CACHE = {}


def _banded(n_out, n_in, pad_in_to=None, pad_out_to=None):
    """R^T for the window-3 stride-2 row sum: [n_in(, padded), n_out] bf16."""
    import ml_dtypes

    r = np.zeros((n_out, n_in), dtype=np.float32)
    for i in range(n_out):
        r[i, 2 * i : 2 * i + 3] = 1.0
    bt = np.ascontiguousarray(r.T)
    if pad_in_to is not None and pad_in_to > n_in:
        bt = np.concatenate(
            [bt, np.zeros((pad_in_to - n_in, n_out), dtype=np.float32)], axis=0
        )
    if pad_out_to is not None and pad_out_to > n_out:
        bt = np.concatenate(
            [bt, np.zeros((bt.shape[0], pad_out_to - n_out), dtype=np.float32)],
            axis=1,
        )
    return bt.astype(ml_dtypes.bfloat16)


def _build_nc():
    from contextlib import ExitStack

    import concourse.bacc as bacc
    import concourse.mybir as mybir
    import concourse.tile as tile

    f32 = mybir.dt.float32
    bf16 = mybir.dt.bfloat16
    SUB = mybir.AluOpType.subtract
    ADD = mybir.AluOpType.add
    AF = mybir.ActivationFunctionType

    nc = bacc.Bacc("TRN2", target_bir_lowering=False, debug=False)
    xys = nc.dram_tensor("xys", [S, P, 2, G0, N0], f32, kind="ExternalInput").ap()
    bt0 = nc.dram_tensor("bt0", [512, 256], bf16, kind="ExternalInput").ap()
    bt1 = nc.dram_tensor("bt1", [256, 128], bf16, kind="ExternalInput").ap()
    bt2 = nc.dram_tensor("bt2", [N2, 64], bf16, kind="ExternalInput").ap()
    bt3 = nc.dram_tensor("bt3", [N3, 32], bf16, kind="ExternalInput").ap()
    sd_out = nc.dram_tensor("sd", [P, 2 * S], f32, kind="ExternalOutput").ap()
    sa_out = nc.dram_tensor("sa", [P, ACT_COLS], f32, kind="ExternalOutput").ap()

    with tile.TileContext(nc) as tc, ExitStack() as ctx:
        singles = ctx.enter_context(tc.tile_pool(name="singles", bufs=1))
        xy = ctx.enter_context(tc.tile_pool(name="xy", bufs=4))
        dpool = ctx.enter_context(tc.tile_pool(name="d", bufs=2))
        vpool = ctx.enter_context(tc.tile_pool(name="v", bufs=4))
        upool = ctx.enter_context(tc.tile_pool(name="u", bufs=2))
        scr = ctx.enter_context(tc.tile_pool(name="scr", bufs=2))
        psA = ctx.enter_context(tc.tile_pool(name="psA", bufs=2, space="PSUM"))
        psum = ctx.enter_context(tc.tile_pool(name="ps", bufs=1, space="PSUM"))

        # banded constants via SWDGE (off the critical qSP ring)
        bt0_sb = singles.tile([P, G0, 256], bf16)
        nc.gpsimd.dma_start(out=bt0_sb, in_=bt0.rearrange("(p g) i -> p g i", g=G0))
        bt1_sb = singles.tile([P, 2, 128], bf16)
        nc.gpsimd.dma_start(out=bt1_sb, in_=bt1.rearrange("(g p) i -> p g i", p=P))
        bt2_sb = singles.tile([N2, 64], bf16)
        nc.gpsimd.dma_start(out=bt2_sb, in_=bt2)
        bt3_sb = singles.tile([N3, 32], bf16)
        nc.gpsimd.dma_start(out=bt3_sb, in_=bt3)

        stats_d = singles.tile([P, 2 * S], f32)    # sd halves via DVE accum
        stats_a = singles.tile([P, ACT_COLS], f32)  # |.| sums via ACT accum
        nc.vector.memset(stats_d, 0.0)
        nc.scalar.memzero(stats_a)

        EVEN = slice(0, 512, 2)

        def colsum_dve(v_out, src, n_out, tag):
            """v_out[..., i] = src[..., 2i]+src[..., 2i+1]+src[..., 2i+2]
            on DVE for a PSUM source.  tensor_tensor may read only ONE
            operand from PSUM, so the odd columns are evacuated to SBUF
            first and each add pairs one PSUM stream with one SBUF one."""
            ps_shape = list(src.shape[:-1]) + [n_out]
            sl = [slice(None)] * (len(src.shape) - 1)
            e0 = src[tuple(sl + [slice(0, 2 * n_out - 1, 2)])]
            e1 = src[tuple(sl + [slice(1, 2 * n_out, 2)])]
            e2 = src[tuple(sl + [slice(2, 2 * n_out + 1, 2)])]
            odd = upool.tile(ps_shape, bf16, tag=f"o{tag}")
            with nc.allow_low_precision(reason="2e-2 tolerance; bf16 pyramid"):
                nc.vector.tensor_copy(out=odd, in_=e1)
            u = upool.tile(ps_shape, bf16, tag=f"u{tag}")
            nc.vector.tensor_add(out=u, in0=e0, in1=odd)
            nc.vector.tensor_add(out=v_out, in0=u, in1=e2)

        pend = {}

        def deep_chain(pr):
            """Levels 1-4 for pair pr, reading level-0 matmuls from PSUM."""
            pdA, pdB = pend.pop(pr)
            s0 = 2 * pr
            # |d1| sums on ACT (per sample, per row-block)
            asc1 = scr.tile([P, 2, 256], bf16, tag="asc1")
            for j in range(2):
                nc.scalar.activation(
                    out=asc1[:, j, 0:N1], in_=pdA[:, j, 0:N1], func=AF.Abs,
                    accum_out=stats_a[:, 8 + s0 + j : 9 + s0 + j],
                )
                nc.scalar.activation(
                    out=asc1[0:127, j, 0:N1], in_=pdB[:, j, 0:N1], func=AF.Abs,
                    accum_out=stats_a[0:127, 16 + s0 + j : 17 + s0 + j],
                )
            # level-1 column sums (DVE, PSUM f32 in -> bf16 out)
            v1a = vpool.tile([P, 2, 128], bf16, tag="v1a")
            v1b = vpool.tile([127, 2, 128], bf16, tag="v1b")
            colsum_dve(v1a[:, :, 0:N2], pdA[:, :, 0:N1], N2, "1a")
            colsum_dve(v1b[:, :, 0:N2], pdB[:, :, 0:N1], N2, "1b")

            # level-2 matmul: d2 = R1 @ v1
            wb2 = psum.tile([N2, 2, 128], f32, tag="wb2")
            for j in range(2):
                nc.tensor.matmul(
                    wb2[:, j, 0:N2], bt1_sb[:, 0, 0:N2], v1a[:, j, 0:N2],
                    start=True, stop=False,
                )
                nc.tensor.matmul(
                    wb2[:, j, 0:N2], bt1_sb[0:127, 1, 0:N2], v1b[:, j, 0:N2],
                    start=False, stop=True,
                )
            asc2 = scr.tile([N2, 2, 128], bf16, tag="asc2")
            for j in range(2):
                nc.scalar.activation(
                    out=asc2[:, j, 0:N2], in_=wb2[:, j, 0:N2], func=AF.Abs,
                    accum_out=stats_a[0:N2, 24 + s0 + j : 25 + s0 + j],
                )
            v2 = vpool.tile([N2, 2, 64], bf16, tag="v2")
            colsum_dve(v2[:, :, 0:N3], wb2[:, :, 0:N2], N3, "2")

            # level-3
            wb3 = psum.tile([N3, 2, 64], f32, tag="wb3")
            for j in range(2):
                nc.tensor.matmul(
                    wb3[:, j, 0:N3], bt2_sb[:, 0:N3], v2[:, j, 0:N3],
                    start=True, stop=True,
                )
            nc.vector.tensor_reduce(
                out=stats_a[0:N3, 32 + s0 : 34 + s0], in_=wb3[:, :, 0:N3],
                axis=mybir.AxisListType.X, op=ADD, apply_absolute_value=True,
            )
            v3 = vpool.tile([N3, 2, 32], bf16, tag="v3")
            colsum_dve(v3[:, :, 0:N4], wb3[:, :, 0:N3], N4, "3")

            # level-4
            wb4 = psum.tile([N4, 2, 32], f32, tag="wb4")
            for j in range(2):
                nc.tensor.matmul(
                    wb4[:, j, 0:N4], bt3_sb[:, 0:N4], v3[:, j, 0:N4],
                    start=True, stop=True,
                )
            nc.vector.tensor_reduce(
                out=stats_a[0:N4, 40 + s0 : 42 + s0], in_=wb4[:, :, 0:N4],
                axis=mybir.AxisListType.X, op=ADD, apply_absolute_value=True,
            )

        for s in range(S):
            xyt = xy.tile([P, 2, G0, N0], f32, tag="xyt")
            nc.sync.dma_start(out=xyt, in_=xys[s])

            if s % 2 == 0:
                pdA = psA.tile([P, 2, 256], f32, tag="pdA")
                pdB = psA.tile([127, 2, 256], f32, tag="pdB")
                pend[s // 2] = (pdA, pdB)

            # d = x - y as packed bf16 even/odd planes; signed sums -> stats_d
            dt = dpool.tile([P, G0, 2, 256], bf16, tag="dt")
            nc.vector.scalar_tensor_tensor(
                out=dt[:, :, 0, :], in0=xyt[:, 0, :, EVEN], scalar=0.0,
                in1=xyt[:, 1, :, EVEN], op0=ADD, op1=SUB,
                accum_out=stats_d[:, 2 * s : 2 * s + 1],
            )
            ODD = slice(1, 512, 2)
            nc.vector.scalar_tensor_tensor(
                out=dt[:, :, 1, :], in0=xyt[:, 0, :, ODD], scalar=0.0,
                in1=xyt[:, 1, :, ODD], op0=ADD, op1=SUB,
                accum_out=stats_d[:, 2 * s + 1 : 2 * s + 2],
            )

            # sum |d| on ACT: one pass over both planes
            ascr = scr.tile([P, G0, 2, 256], bf16, tag="ascr")
            nc.scalar.activation(
                out=ascr, in_=dt, func=AF.Abs,
                accum_out=stats_a[:, s : s + 1],
            )

            # level-0 column-window sum:
            #   u = e + o (DVE 2x packed), v0 = u + e[1:] (GpSimd)
            u0 = upool.tile([P, G0, 256], bf16, tag="u0")
            nc.vector.tensor_add(
                out=u0[:, :, 0:N1], in0=dt[:, :, 0, 0:N1], in1=dt[:, :, 1, 0:N1]
            )
            v0 = vpool.tile([P, G0, 256], bf16, tag="v0")
            if s < S - 2:
                nc.gpsimd.tensor_add(
                    out=v0[:, :, 0:N1], in0=u0[:, :, 0:N1], in1=dt[:, :, 0, 1:256]
                )
            else:
                # tail samples: keep the critical path off the slow Q7
                nc.vector.tensor_add(
                    out=v0[:, :, 0:N1], in0=u0[:, :, 0:N1], in1=dt[:, :, 0, 1:256]
                )

            # level-0 row-window matmuls, emitted per sample.  Row r =
            # 4p+g: block m=0 (rows 0..127) draws from partitions [0:65]
            # (g=0) / [0:64] (g>0); m=1 (rows 128..254) from [64:128].
            pdA, pdB = pend[s // 2]
            for w, mp, m in ((pdA, P, 0), (pdB, 127, 1)):
                for g in range(G0):
                    if m == 0:
                        lo, hi = 0, (65 if g == 0 else 64)
                    else:
                        lo, hi = 64, 128
                    nc.tensor.matmul(
                        w[:, s % 2, 0:N1],
                        bt0_sb[lo:hi, g, m * 128 : m * 128 + mp],
                        v0[lo:hi, g, 0:N1],
                        start=(g == 0),
                        stop=(g == G0 - 1),
                    )

            # deep-chain one pair behind so PE latency never blocks DVE
            if s % 2 == 1 and s >= 3:
                deep_chain(s // 2 - 1)

        deep_chain(S // 2 - 1)

        nc.sync.dma_start(out=sd_out, in_=stats_d)
        nc.sync.dma_start(out=sa_out, in_=stats_a)

    nc.finalize()
    return nc


def _get_nc():
    if "nc" not in _CACHE:
        _CACHE["nc"] = _build_nc()
    return _CACHE["nc"]


def _run_on_hw(x, y, trace=False):
    """x, y: [64, 512, 512] fp32 numpy. Returns list of 8 (sd, sa) pairs."""
    from concourse.bass_utils import run_bass_kernel_spmd

    nc = _get_nc()
    bt0 = _banded(N1, 512, pad_out_to=256)
    bt1 = _banded(N2, N1, pad_in_to=256, pad_out_to=128)
    bt2 = _banded(N3, N2, pad_out_to=64)
    bt3 = _banded(N4, N3, pad_out_to=32)

    in_maps = []
    for c in range(NCORES):
        xc = x[c * S : (c + 1) * S]
        yc = y[c * S : (c + 1) * S]
        xb = xc.reshape(S, P, G0, N0)
        yb = yc.reshape(S, P, G0, N0)
        in_maps.append(
            {
                "xys": np.ascontiguousarray(np.stack([xb, yb], axis=2)),
                "bt0": bt0,
                "bt1": bt1,
                "bt2": bt2,
                "bt3": bt3,
            }
        )

    res = run_bass_kernel_spmd(
        nc, in_maps, core_ids=list(range(NCORES)), trace=trace
    )
    _CACHE["last_results"] = res
    return [(r["sd"], r["sa"]) for r in res.results]


def kernel(x, y, alpha, _trace=False):
    x = np.ascontiguousarray(np.asarray(x, dtype=np.float32).reshape(B, 512, 512))
    y = np.ascontiguousarray(np.asarray(y, dtype=np.float32).reshape(B, 512, 512))
    alpha = np.asarray(alpha, dtype=np.float32)

    stats_list = _run_on_hw(x, y, trace=_trace)

    sd = np.empty(B, np.float64)
    sa = np.empty((B, 5), np.float64)
    for c in range(NCORES):
        std, sta = stats_list[c]
        std = std.astype(np.float64)
        sta = sta.astype(np.float64)
        for s in range(S):
            b = c * S + s
            sd[b] = std[:, 2 * s].sum() + std[:, 2 * s + 1].sum()
            sa[b, 0] = sta[:, s].sum()
            sa[b, 1] = sta[:, 8 + s].sum() + sta[0:127, 16 + s].sum()
            sa[b, 2] = sta[0:N2, 24 + s].sum()
            sa[b, 3] = sta[0:N3, 32 + s].sum()
            sa[b, 4] = sta[0:N4, 40 + s].sum()

    counts = np.array(
        [N0 * N0, N1 * N1, N2 * N2, N3 * N3, N4 * N4], np.float64
    )
    l1 = sa / counts  # [B, 5]
    last = np.abs(sd) * float(LAYER_NUM + 1)  # [B]

    # faithful 'fuhao' replication (matches reference.py exactly)
    k_layer = (alpha * np.float32(LAYER_NUM + 2)).astype(np.int32)  # [B]
    trig = k_layer <= LAYER_NUM
    triggered_before = np.concatenate(
        [np.zeros(1, bool), np.cumsum(trig)[:-1] > 0]
    )
    i_idx = np.arange(LAYER_NUM + 1)
    sign = np.where(
        triggered_before[:, None] | (i_idx[None, :] >= k_layer[:, None]),
        1.0,
        -1.0,
    )

    loss_tensor = np.concatenate([l1 * sign, last[:, None]], axis=1)
    return np.float32(loss_tensor.mean())
